# revision 3
# baseline (speedup 1.0000x reference)
"""Two-layer GCN (PyG GCNConv x2 + ReLU) on 8 Trainium2 NeuronCores — v2.

Strategy (dst-sharded SPMD, fp16 data path, on-chip selection matrices):
  - Nodes padded to 102400, sharded 12800/core by destination via a
    degree-balanced permutation; 128x128 weights replicated.
  - Per layer: dense h = x_shard @ W (fp16 in, fp32 psum) -> quartered
    fp16 AllGather (window w = source-quarter w of every core's shard, so
    gathers can chase collective pieces) -> windowed dma_gather of h[src]
    (int16 idx) -> scatter-add via per-tile selection matrices
    S[e, dst] = (iota[d]==dloc[e]) * norm[e], built ON-CHIP with one DVE
    tensor_scalar per 128-edge tile (no S streaming from HBM) -> fp16
    128x128 matmuls accumulating per (window, superblock) into a [128,512]
    PSUM bank, added into an SBUF fp32 accumulator per superblock.
  - Self-loop contributions skip the gather: the dense output stays in
    SBUF and a diagonal S (same tensor_scalar trick) initializes the
    accumulator while layer-1's collective is still in flight.
  - Both layers accumulate transposed [f, dst]. Layer-1 epilogue
    relu(acc+b1) directly emits the fp16 lhsT for layer 2's dense matmul;
    layer-2 epilogue does relu(+b2), a PE transpose back to [node, f], and
    writes fp32 rows. Layer-2's collective pieces fire during layer-1's
    window-3 processing, hiding them behind the gather stream.
"""

import numpy as np

import concourse.bass as bass
import concourse.bacc as bacc
import concourse.mybir as mybir
import concourse.tile as tile
from concourse.bass_utils import run_bass_kernel_spmd

N = 100000
E = 640000
D = 128
NCORES = 8
NPAD = 102400
SHARD = NPAD // NCORES        # 12800
NBLK = SHARD // 128           # 100 dst blocks per core
SB = 4                        # dst blocks per superblock (one 2KB PSUM bank)
NSB = NBLK // SB              # 25 superblocks
NW = 4                        # gather windows == source quarters
QROWS = SHARD // NW           # 3200 bounce rows per collective piece
WIN = QROWS * NCORES          # 25600 rows per window table
CHUNK_T = 8                   # tiles per dma_gather call (1024 idx ring limit)
NGRP = NW * NBLK              # (w, s, bi) groups, window-major
NQ = 4                        # collective pieces per layer
JPQ = NBLK // NQ              # dense blocks per collective piece

f16 = mybir.dt.float16
f32 = mybir.dt.float32
i16 = mybir.dt.int16

# bisect flags (timing experiments; correctness breaks when skipping)
SKIP_CC = False
SKIP_GATHER = False
SKIP_EDGE = False

_CACHE = {}


def _host_prep(x, edge_index, W1, b1, W2, b2):
    x = np.asarray(x, dtype=np.float32)
    ei = np.asarray(edge_index)
    W1 = np.asarray(W1, dtype=np.float32)
    W2 = np.asarray(W2, dtype=np.float32)
    b1 = np.asarray(b1, dtype=np.float32)
    b2 = np.asarray(b2, dtype=np.float32)
    n = x.shape[0]

    src = ei[0].astype(np.int64)
    dst = ei[1].astype(np.int64)
    deg = np.bincount(np.concatenate([dst, np.arange(n, dtype=np.int64)]),
                      minlength=NPAD).astype(np.float32)
    a = np.zeros(NPAD, np.float32)
    nz = deg > 0
    a[nz] = 1.0 / np.sqrt(deg[nz])

    # degree-balanced node->position permutation
    order_by_deg = np.argsort(-deg, kind="stable")
    i = np.arange(NPAD, dtype=np.int64)
    cb = i % (NCORES * NBLK)
    position_of_rank = (cb % NCORES) * SHARD + (cb // NCORES) * 128 + i // (NCORES * NBLK)
    pos_of_node = np.empty(NPAD, np.int64)
    pos_of_node[order_by_deg] = position_of_rank
    node_at_pos = np.empty(NPAD, np.int64)
    node_at_pos[pos_of_node] = np.arange(NPAD, dtype=np.int64)

    ps = pos_of_node[src]
    pd = pos_of_node[dst]
    core = pd // SHARD
    # window = source quarter (AllGather piece q = rows [q*QROWS,(q+1)*QROWS)
    # of every core's shard, concatenated by core)
    w_e = (ps % SHARD) // QROWS
    row_e = (ps // SHARD) * QROWS + (ps % SHARD) - w_e * QROWS
    b_e = (pd % SHARD) // 128
    g_e = w_e * NBLK + b_e            # window-major group id
    dloc_e = pd % 128
    norm_e = a[src] * a[dst]

    per_core = []
    counts_all = np.zeros((NCORES, NGRP), np.int64)
    for k in range(NCORES):
        m = core == k
        g_k, row_k, dloc_k, norm_k = g_e[m], row_e[m], dloc_e[m], norm_e[m]
        order = np.lexsort((row_k, g_k))
        g_k, row_k, dloc_k, norm_k = (g_k[order], row_k[order],
                                      dloc_k[order], norm_k[order])
        counts_all[k] = np.bincount(g_k, minlength=NGRP)
        per_core.append((g_k, row_k, dloc_k, norm_k))

    # every group gets >=1 tile so each PSUM slice is initialized (pad tiles
    # carry norm=0 and are harmless)
    T = np.maximum((np.max(counts_all, axis=0) + 127) // 128, 1)
    tile_base = np.zeros(NGRP + 1, np.int64)
    tile_base[1:] = np.cumsum(T)
    t_total = int(tile_base[-1])

    # tile -> (block-in-superblock, start, stop) and per-(w,s) tile ranges
    tinfo = []
    for g in range(NGRP):
        bi = g % SB
        for t in range(int(tile_base[g]), int(tile_base[g + 1])):
            tinfo.append((bi, t == int(tile_base[g]),
                          t == int(tile_base[g + 1]) - 1))
    ws_range = {}
    for w in range(NW):
        for s in range(NSB):
            g0 = w * NBLK + s * SB
            ws_range[(w, s)] = (int(tile_base[g0]), int(tile_base[g0 + SB]))
    calls = []  # (w, s_of_call_start, t0, nt): chunks within a window
    for w in range(NW):
        for s in range(NSB):
            t0, t_end = ws_range[(w, s)]
            t = t0
            while t < t_end:
                nt = min(CHUNK_T, t_end - t)
                calls.append((w, s, t, nt))
                t += nt

    x_pad = np.zeros((NPAD, D), np.float32)
    x_pad[:n] = x
    x_perm = x_pad[node_at_pos]
    a2_pos = (a[node_at_pos] ** 2).astype(np.float32)

    iota32 = np.tile(np.arange(128, dtype=np.float16), (128, 1))
    ident16 = np.eye(128, dtype=np.float16)
    dcol32 = np.arange(128, dtype=np.float32).reshape(128, 1)

    in_maps = []
    for k in range(NCORES):
        g_k, row_k, dloc_k, norm_k = per_core[k]
        ne = g_k.shape[0]
        grp_off = np.zeros(NGRP + 1, np.int64)
        grp_off[1:] = np.cumsum(counts_all[k])
        rank = np.arange(ne, dtype=np.int64) - grp_off[g_k]
        slot = tile_base[g_k] * 128 + rank

        gidx = np.zeros(t_total * 128, np.int16)
        gidx[slot] = row_k.astype(np.int16)
        dlocA = np.zeros(t_total * 128, np.float32)
        dlocA[slot] = dloc_k.astype(np.float32)
        normA = np.zeros(t_total * 128, np.float32)
        normA[slot] = norm_k
        dloc32 = np.ascontiguousarray(dlocA.reshape(t_total, 128).T)
        norm32 = np.ascontiguousarray(normA.reshape(t_total, 128).T)

        idxw = np.zeros((128, t_total * 8), np.int16)
        for (w, s, t0, nt) in calls:
            blk = gidx[t0 * 128:(t0 + nt) * 128].reshape(nt * 8, 16).T
            idxw[:, t0 * 8:(t0 + nt) * 8] = np.tile(blk, (8, 1))

        snorm32 = np.ascontiguousarray(
            a2_pos[k * SHARD:(k + 1) * SHARD].reshape(NBLK, 128).T)
        xT16 = np.ascontiguousarray(
            x_perm[k * SHARD:(k + 1) * SHARD].T).astype(np.float16)

        in_maps.append({
            "xT16": xT16,
            "W1_16": W1.astype(np.float16),
            "W2_16": W2.astype(np.float16),
            "b1col": b1.reshape(128, 1).copy(),
            "b2col": b2.reshape(128, 1).copy(),
            "iota32": iota32,
            "ident16": ident16,
            "dcol32": dcol32,
            "snorm32": snorm32,
            "dloc32": dloc32,
            "norm32": norm32,
            "idxw": idxw,
        })

    sched_sig = tuple(int(v) for v in T)
    return (in_maps, sched_sig, tuple(int(v) for v in tile_base), t_total,
            tinfo, ws_range, calls, pos_of_node)


def _build_program(tile_base, t_total, tinfo, ws_range, calls):
    nc = bacc.Bacc("TRN2", target_bir_lowering=False, debug=False,
                   num_devices=NCORES, num_swdge_queues=4)
    xT_d = nc.dram_tensor("xT16", [D, SHARD], f16, kind="ExternalInput")
    W1_d = nc.dram_tensor("W1_16", [D, D], f16, kind="ExternalInput")
    W2_d = nc.dram_tensor("W2_16", [D, D], f16, kind="ExternalInput")
    b1_d = nc.dram_tensor("b1col", [128, 1], f32, kind="ExternalInput")
    b2_d = nc.dram_tensor("b2col", [128, 1], f32, kind="ExternalInput")
    iota_d = nc.dram_tensor("iota32", [128, 128], f16, kind="ExternalInput")
    ident_d = nc.dram_tensor("ident16", [128, 128], f16, kind="ExternalInput")
    dcol_d = nc.dram_tensor("dcol32", [128, 1], f32, kind="ExternalInput")
    snorm_d = nc.dram_tensor("snorm32", [128, NBLK], f32, kind="ExternalInput")
    dloc_d = nc.dram_tensor("dloc32", [128, t_total], f32, kind="ExternalInput")
    norm_d = nc.dram_tensor("norm32", [128, t_total], f32, kind="ExternalInput")
    idx_d = nc.dram_tensor("idxw", [128, t_total * 8], i16, kind="ExternalInput")
    out_d = nc.dram_tensor("out", [SHARD, D], f32, kind="ExternalOutput")

    h_bq = [[nc.dram_tensor(f"h{l}_bq{q}", [QROWS, D], f16) for q in range(NQ)]
            for l in range(2)]
    h_w = [[nc.dram_tensor(f"h{l}_w{q}", [WIN, D], f16, addr_space="Shared")
            for q in range(NQ)] for l in range(2)]

    with tile.TileContext(nc) as tc:
        with (
            tc.tile_pool(name="const", bufs=1) as p_const,
            tc.tile_pool(name="big", bufs=1) as p_big,
            tc.tile_pool(name="msg", bufs=12) as p_msg,
            tc.tile_pool(name="sel", bufs=16) as p_sel,
            tc.tile_pool(name="r16", bufs=2) as p_r16,
            tc.tile_pool(name="o32", bufs=4) as p_o32,
            tc.tile_pool(name="wps", bufs=3, space="PSUM") as p_wps,
            tc.tile_pool(name="dps", bufs=2, space="PSUM") as p_dps,
            tc.tile_pool(name="tps", bufs=2, space="PSUM") as p_tps,
        ):
            W1_t = p_const.tile([D, D], f16)
            W2_t = p_const.tile([D, D], f16)
            b1_t = p_const.tile([128, 1], f32)
            b2_t = p_const.tile([128, 1], f32)
            iota_t = p_const.tile([128, 128], f16)
            ident_t = p_const.tile([128, 128], f16)
            dcol_t = p_const.tile([128, 1], f32)
            snorm_t = p_const.tile([128, NBLK], f32)
            dloc_t = p_const.tile([128, t_total], f32)
            norm_t = p_const.tile([128, t_total], f32)
            idx_t = p_const.tile([128, t_total * 8], i16)
            xT_t = p_const.tile([D, SHARD], f16)
            relu1_t = p_big.tile([128, SHARD], f16, tag="relu1")
            for tt, dd in ((W1_t, W1_d), (W2_t, W2_d), (b1_t, b1_d),
                           (b2_t, b2_d), (iota_t, iota_d), (ident_t, ident_d),
                           (dcol_t, dcol_d), (snorm_t, snorm_d),
                           (dloc_t, dloc_d), (norm_t, norm_d), (idx_t, idx_d),
                           (xT_t, xT_d)):
                nc.sync.dma_start(out=tt[:], in_=dd[:])

            def build_S(out_ap, scalar1, scalar2):
                nc.vector.tensor_scalar(
                    out=out_ap, in0=iota_t[:], scalar1=scalar1, scalar2=scalar2,
                    op0=mybir.AluOpType.is_equal, op1=mybir.AluOpType.mult)

            def dense_block(lhsT_full, W_t, hloc_t, l, j):
                ps = p_dps.tile([128, D], f32, space="PSUM", tag="dps")
                nc.tensor.matmul(out=ps[:],
                                 lhsT=lhsT_full[:, j * 128:(j + 1) * 128],
                                 rhs=W_t[:], start=True, stop=True)
                hsl = hloc_t[:, j * 128:(j + 1) * 128]
                nc.scalar.activation(out=hsl, in_=ps[:],
                                     func=mybir.ActivationFunctionType.Copy)
                q, jr = j // JPQ, j % JPQ
                nc.sync.dma_start(out=h_bq[l][q][jr * 128:(jr + 1) * 128, :],
                                  in_=hsl)
                if (j + 1) % JPQ == 0 and not SKIP_CC:
                    nc.gpsimd.collective_compute(
                        "AllGather", mybir.AluOpType.bypass,
                        replica_groups=[list(range(NCORES))],
                        ins=[h_bq[l][q][:]], outs=[h_w[l][q][:]])

            msg0 = None
            if SKIP_GATHER:
                msg0 = p_const.tile([128, CHUNK_T, D], f16)
                nc.vector.memset(msg0[:], 0.5)

            def edge_phase(l, hloc_t, acc, epilogue_cb):
                # self phase: diagonal S from the SBUF dense output
                for s in range(NSB):
                    pw = p_wps.tile([128, SB * 128], f32, space="PSUM",
                                    tag="wps")
                    for bi in range(SB):
                        b = s * SB + bi
                        Ssf = p_sel.tile([128, 128], f16, tag="sel")
                        build_S(Ssf[:], dcol_t[:, :1], snorm_t[:, b:b + 1])
                        nc.tensor.matmul(
                            out=pw[:, bi * 128:(bi + 1) * 128],
                            lhsT=hloc_t[:, b * 128:(b + 1) * 128], rhs=Ssf[:],
                            start=True, stop=True, skip_group_check=True)
                    nc.scalar.activation(
                        out=acc[:, s * SB * 128:(s + 1) * SB * 128],
                        in_=pw[:], func=mybir.ActivationFunctionType.Copy)
                # window phases, chasing the collective pieces
                call_i = 0
                for w in range(NW):
                    s_open = -1
                    pw = None
                    for s in range(NSB):
                        t0, t_end = ws_range[(w, s)]
                        if t0 == t_end:
                            if w == NW - 1:
                                epilogue_cb(s)
                            continue
                        while call_i < len(calls) and calls[call_i][2] < t_end \
                                and calls[call_i][0] == w:
                            _, _, c0, cnt = calls[call_i]
                            if SKIP_GATHER:
                                msg = msg0
                            else:
                              msg = p_msg.tile([128, CHUNK_T, D], f16, tag="msg")
                              nc.gpsimd.dma_gather(
                                out_ap=msg[:, :cnt, :], in_ap=h_w[l][w][:],
                                idxs_ap=idx_t[:, c0 * 8:(c0 + cnt) * 8],
                                num_idxs=cnt * 128, num_idxs_reg=cnt * 128,
                                elem_size=D, queue_num=call_i % 4)
                            call_i += 1
                            for t in range(c0, c0 + cnt):
                                bi_t, start_t, stop_t = tinfo[t]
                                if SKIP_EDGE:
                                    if t == t_end - 1 and w == NW - 1:
                                        epilogue_cb(s)
                                    continue
                                if t == t0:
                                    pw = p_wps.tile([128, SB * 128], f32,
                                                    space="PSUM", tag="wps")
                                    s_open = s
                                St = p_sel.tile([128, 128], f16, tag="sel")
                                build_S(St[:], dloc_t[:, t:t + 1],
                                        norm_t[:, t:t + 1])
                                nc.tensor.matmul(
                                    out=pw[:, bi_t * 128:(bi_t + 1) * 128],
                                    lhsT=msg[:, t - c0, :], rhs=St[:],
                                    start=start_t, stop=stop_t,
                                    skip_group_check=True)
                                if t == t_end - 1:
                                    sl = acc[:, s * SB * 128:(s + 1) * SB * 128]
                                    nc.vector.tensor_add(out=sl, in0=sl,
                                                         in1=pw[:])
                                    if w == NW - 1:
                                        epilogue_cb(s)
                        # calls list is window-major so the inner while covers
                        # every superblock of this window in order

            # ---------- layer 1 ----------
            hloc_t = p_big.tile([128, SHARD], f16, tag="hloc")
            for j in range(NBLK):
                dense_block(xT_t, W1_t, hloc_t, 0, j)

            acc1 = p_big.tile([128, SHARD], f32, tag="acc")
            hloc2_t = None

            def epi1(s):
                nonlocal hloc2_t
                nc.scalar.activation(
                    out=relu1_t[:, s * 512:(s + 1) * 512],
                    in_=acc1[:, s * 512:(s + 1) * 512],
                    func=mybir.ActivationFunctionType.Relu, bias=b1_t[:, :1])
                for bi in range(SB):
                    b = s * SB + bi
                    dense_block(relu1_t, W2_t, hloc2_t, 1, b)

            hloc2_t = p_big.tile([128, SHARD], f16, tag="hloc")
            edge_phase(0, hloc_t, acc1, epi1)

            # ---------- layer 2 ----------
            acc2 = p_big.tile([128, SHARD], f32, tag="acc")

            def epi2(s):
                r16 = p_r16.tile([128, 512], f16, tag="r16")
                nc.scalar.activation(
                    out=r16[:], in_=acc2[:, s * 512:(s + 1) * 512],
                    func=mybir.ActivationFunctionType.Relu, bias=b2_t[:, :1])
                for bi in range(SB):
                    b = s * SB + bi
                    tp = p_tps.tile([128, 128], f16, space="PSUM", tag="tps")
                    nc.tensor.transpose(tp[:], r16[:, bi * 128:(bi + 1) * 128],
                                        ident_t[:])
                    o32 = p_o32.tile([128, 128], f32, tag="o32")
                    nc.scalar.activation(out=o32[:], in_=tp[:],
                                         func=mybir.ActivationFunctionType.Copy)
                    nc.sync.dma_start(out=out_d[b * 128:(b + 1) * 128, :],
                                      in_=o32[:])

            edge_phase(1, hloc2_t, acc2, epi2)

    nc.compile()
    return nc


def prepare(x, edge_index, W1, b1, W2, b2):
    (in_maps, sched_sig, tile_base, t_total, tinfo, ws_range, calls,
     pos_of_node) = _host_prep(x, edge_index, W1, b1, W2, b2)
    key = (sched_sig, SKIP_CC, SKIP_GATHER, SKIP_EDGE)
    if key not in _CACHE:
        _CACHE[key] = _build_program(tile_base, t_total, tinfo, ws_range,
                                     calls)
    return _CACHE[key], in_maps, pos_of_node


def kernel(x, edge_index, W1, b1, W2, b2):
    nc, in_maps, pos_of_node = prepare(x, edge_index, W1, b1, W2, b2)
    res = run_bass_kernel_spmd(nc, in_maps, list(range(NCORES)))
    full = np.concatenate([res.results[k]["out"] for k in range(NCORES)], axis=0)
    n = np.asarray(x).shape[0]
    return full[pos_of_node[:n]]


# revision 4
# speedup vs baseline: 2.0715x; 2.0715x over previous
"""Two-layer GCN (PyG GCNConv x2 + ReLU) on 8 Trainium2 NeuronCores — v2.

Strategy (dst-sharded SPMD, fp16 data path, on-chip selection matrices):
  - Nodes padded to 102400, sharded 12800/core by destination via a
    degree-balanced permutation; 128x128 weights replicated.
  - Per layer: dense h = x_shard @ W (fp16 in, fp32 psum) -> quartered
    fp16 AllGather (window w = source-quarter w of every core's shard, so
    gathers can chase collective pieces) -> windowed dma_gather of h[src]
    (int16 idx) -> scatter-add via per-tile selection matrices
    S[e, dst] = (iota[d]==dloc[e]) * norm[e], built ON-CHIP with one DVE
    tensor_scalar per 128-edge tile (no S streaming from HBM) -> fp16
    128x128 matmuls accumulating per (window, superblock) into a [128,512]
    PSUM bank, added into an SBUF fp32 accumulator per superblock.
  - Self-loop contributions skip the gather: the dense output stays in
    SBUF and a diagonal S (same tensor_scalar trick) initializes the
    accumulator while layer-1's collective is still in flight.
  - Both layers accumulate transposed [f, dst]. Layer-1 epilogue
    relu(acc+b1) directly emits the fp16 lhsT for layer 2's dense matmul;
    layer-2 epilogue does relu(+b2), a PE transpose back to [node, f], and
    writes fp32 rows. Layer-2's collective pieces fire during layer-1's
    window-3 processing, hiding them behind the gather stream.
"""

import numpy as np

import concourse.bass as bass
import concourse.bacc as bacc
import concourse.mybir as mybir
import concourse.tile as tile
from concourse.bass_utils import run_bass_kernel_spmd

N = 100000
E = 640000
D = 128
NCORES = 8
NPAD = 102400
SHARD = NPAD // NCORES        # 12800
NBLK = SHARD // 128           # 100 dst blocks per core
SB = 4                        # dst blocks per superblock (one 2KB PSUM bank)
NSB = NBLK // SB              # 25 superblocks
NW = 4                        # gather windows == source quarters
QROWS = SHARD // NW           # 3200 bounce rows per collective piece
WIN = QROWS * NCORES          # 25600 rows per window table
CHUNK_T = 8                   # tiles per dma_gather call (1024 idx ring limit)
NGRP = NW * NBLK              # (w, s, bi) groups, window-major
NQ = 4                        # collective pieces per layer
JPQ = NBLK // NQ              # dense blocks per collective piece

f16 = mybir.dt.float16
f32 = mybir.dt.float32
i16 = mybir.dt.int16

# bisect flags (timing experiments; correctness breaks when skipping)
SKIP_CC = False
SKIP_GATHER = False
SKIP_EDGE = False
DEBUG_DUMP = False
SGC = True

_CACHE = {}


def _host_prep(x, edge_index, W1, b1, W2, b2):
    x = np.asarray(x, dtype=np.float32)
    ei = np.asarray(edge_index)
    W1 = np.asarray(W1, dtype=np.float32)
    W2 = np.asarray(W2, dtype=np.float32)
    b1 = np.asarray(b1, dtype=np.float32)
    b2 = np.asarray(b2, dtype=np.float32)
    n = x.shape[0]

    src = ei[0].astype(np.int64)
    dst = ei[1].astype(np.int64)
    deg = np.bincount(np.concatenate([dst, np.arange(n, dtype=np.int64)]),
                      minlength=NPAD).astype(np.float32)
    a = np.zeros(NPAD, np.float32)
    nz = deg > 0
    a[nz] = 1.0 / np.sqrt(deg[nz])

    # degree-balanced node->position permutation
    order_by_deg = np.argsort(-deg, kind="stable")
    i = np.arange(NPAD, dtype=np.int64)
    cb = i % (NCORES * NBLK)
    position_of_rank = (cb % NCORES) * SHARD + (cb // NCORES) * 128 + i // (NCORES * NBLK)
    pos_of_node = np.empty(NPAD, np.int64)
    pos_of_node[order_by_deg] = position_of_rank
    node_at_pos = np.empty(NPAD, np.int64)
    node_at_pos[pos_of_node] = np.arange(NPAD, dtype=np.int64)

    ps = pos_of_node[src]
    pd = pos_of_node[dst]
    core = pd // SHARD
    # window = source quarter (AllGather piece q = rows [q*QROWS,(q+1)*QROWS)
    # of every core's shard, concatenated by core)
    w_e = (ps % SHARD) // QROWS
    row_e = (ps // SHARD) * QROWS + (ps % SHARD) - w_e * QROWS
    b_e = (pd % SHARD) // 128
    g_e = ((b_e // SB) * NW + w_e) * SB + (b_e % SB)  # superblock-major
    dloc_e = pd % 128
    norm_e = a[src] * a[dst]

    per_core = []
    counts_all = np.zeros((NCORES, NGRP), np.int64)
    for k in range(NCORES):
        m = core == k
        g_k, row_k, dloc_k, norm_k = g_e[m], row_e[m], dloc_e[m], norm_e[m]
        order = np.lexsort((row_k, g_k))
        g_k, row_k, dloc_k, norm_k = (g_k[order], row_k[order],
                                      dloc_k[order], norm_k[order])
        counts_all[k] = np.bincount(g_k, minlength=NGRP)
        per_core.append((g_k, row_k, dloc_k, norm_k))

    # every group gets >=1 tile so each PSUM slice is initialized (pad tiles
    # carry norm=0 and are harmless)
    T = np.maximum((np.max(counts_all, axis=0) + 127) // 128, 1)
    tile_base = np.zeros(NGRP + 1, np.int64)
    tile_base[1:] = np.cumsum(T)
    t_total = int(tile_base[-1])

    # per-block last window with tiles (for the PSUM stop flag); with
    # T>=1 everywhere last_w is always NW-1, but keep it general
    last_w = np.full(NBLK, NW - 1, np.int64)
    # tile -> (block-in-superblock, start, stop)
    tinfo = []
    for g in range(NGRP):
        s, w, bi = g // (NW * SB), (g // SB) % NW, g % SB
        b = s * SB + bi
        for t in range(int(tile_base[g]), int(tile_base[g + 1])):
            tinfo.append((bi, False,
                          w == last_w[b] and t == int(tile_base[g + 1]) - 1))
    ws_range = {}
    for w in range(NW):
        for s in range(NSB):
            g0 = (s * NW + w) * SB
            ws_range[(w, s)] = (int(tile_base[g0]), int(tile_base[g0 + SB]))
    calls = []  # (w, s, t0, nt): chunks within an (s, w) region
    for s in range(NSB):
        for w in range(NW):
            t0, t_end = ws_range[(w, s)]
            t = t0
            while t < t_end:
                nt = min(CHUNK_T, t_end - t)
                calls.append((w, s, t, nt))
                t += nt

    x_pad = np.zeros((NPAD, D), np.float32)
    x_pad[:n] = x
    x_perm = x_pad[node_at_pos]
    a2_pos = (a[node_at_pos] ** 2).astype(np.float32)

    iota32 = np.tile(np.arange(128, dtype=np.float16), (128, 1))
    ident16 = np.eye(128, dtype=np.float16)
    dcol32 = np.arange(128, dtype=np.float32).reshape(128, 1)

    in_maps = []
    for k in range(NCORES):
        g_k, row_k, dloc_k, norm_k = per_core[k]
        ne = g_k.shape[0]
        grp_off = np.zeros(NGRP + 1, np.int64)
        grp_off[1:] = np.cumsum(counts_all[k])
        rank = np.arange(ne, dtype=np.int64) - grp_off[g_k]
        slot = tile_base[g_k] * 128 + rank

        gidx = np.zeros(t_total * 128, np.int16)
        gidx[slot] = row_k.astype(np.int16)
        dlocA = np.zeros(t_total * 128, np.float32)
        dlocA[slot] = dloc_k.astype(np.float32)
        normA = np.zeros(t_total * 128, np.float32)
        normA[slot] = norm_k
        dloc32 = np.ascontiguousarray(dlocA.reshape(t_total, 128).T)
        norm32 = np.ascontiguousarray(normA.reshape(t_total, 128).T)

        idxw = np.zeros((128, t_total * 8), np.int16)
        for (w, s, t0, nt) in calls:
            blk = gidx[t0 * 128:(t0 + nt) * 128].reshape(nt * 8, 16).T
            idxw[:, t0 * 8:(t0 + nt) * 8] = np.tile(blk, (8, 1))

        snorm32 = np.ascontiguousarray(
            a2_pos[k * SHARD:(k + 1) * SHARD].reshape(NBLK, 128).T)
        xT16 = np.ascontiguousarray(
            x_perm[k * SHARD:(k + 1) * SHARD].T).astype(np.float16)

        in_maps.append({
            "xT16": xT16,
            "W1_16": W1.astype(np.float16),
            "W2_16": W2.astype(np.float16),
            "b1col": b1.reshape(128, 1).copy(),
            "b2col": b2.reshape(128, 1).copy(),
            "iota32": iota32,
            "ident16": ident16,
            "dcol32": dcol32,
            "snorm32": snorm32,
            "dloc32": dloc32,
            "norm32": norm32,
            "idxw": idxw,
        })

    sched_sig = tuple(int(v) for v in T)
    return (in_maps, sched_sig, tuple(int(v) for v in tile_base), t_total,
            tinfo, ws_range, calls, pos_of_node)


def _build_program(tile_base, t_total, tinfo, ws_range, calls):
    nc = bacc.Bacc("TRN2", target_bir_lowering=False, debug=False,
                   num_devices=NCORES, num_swdge_queues=4)
    xT_d = nc.dram_tensor("xT16", [D, SHARD], f16, kind="ExternalInput")
    W1_d = nc.dram_tensor("W1_16", [D, D], f16, kind="ExternalInput")
    W2_d = nc.dram_tensor("W2_16", [D, D], f16, kind="ExternalInput")
    b1_d = nc.dram_tensor("b1col", [128, 1], f32, kind="ExternalInput")
    b2_d = nc.dram_tensor("b2col", [128, 1], f32, kind="ExternalInput")
    iota_d = nc.dram_tensor("iota32", [128, 128], f16, kind="ExternalInput")
    ident_d = nc.dram_tensor("ident16", [128, 128], f16, kind="ExternalInput")
    dcol_d = nc.dram_tensor("dcol32", [128, 1], f32, kind="ExternalInput")
    snorm_d = nc.dram_tensor("snorm32", [128, NBLK], f32, kind="ExternalInput")
    dloc_d = nc.dram_tensor("dloc32", [128, t_total], f32, kind="ExternalInput")
    norm_d = nc.dram_tensor("norm32", [128, t_total], f32, kind="ExternalInput")
    idx_d = nc.dram_tensor("idxw", [128, t_total * 8], i16, kind="ExternalInput")
    out_d = nc.dram_tensor("out", [SHARD, D], f32, kind="ExternalOutput")

    if DEBUG_DUMP:
        dbg_w = nc.dram_tensor("dbg_w", [128, D], f32, kind="ExternalOutput")
        dbg_r1 = nc.dram_tensor("dbg_r1", [128, SHARD], f32, kind="ExternalOutput")
        dbg_h1 = nc.dram_tensor("dbg_h1", [128, 512], f32, kind="ExternalOutput")
    h_bq = [[nc.dram_tensor(f"h{l}_bq{q}", [QROWS, D], f16) for q in range(NQ)]
            for l in range(2)]
    h_w = [[nc.dram_tensor(f"h{l}_w{q}", [WIN, D], f16, addr_space="Shared")
            for q in range(NQ)] for l in range(2)]

    with tile.TileContext(nc) as tc:
        with (
            tc.tile_pool(name="const", bufs=1) as p_const,
            tc.tile_pool(name="big", bufs=1) as p_big,
            tc.tile_pool(name="msg", bufs=16) as p_msg,
            tc.tile_pool(name="sel", bufs=16) as p_sel,
            tc.tile_pool(name="r16", bufs=2) as p_r16,
            tc.tile_pool(name="o32", bufs=4) as p_o32,
            tc.tile_pool(name="wps", bufs=3, space="PSUM") as p_wps,
            tc.tile_pool(name="dps", bufs=2, space="PSUM") as p_dps,
            tc.tile_pool(name="tps", bufs=2, space="PSUM") as p_tps,
        ):
            W1_t = p_const.tile([D, D], f16)
            W2_t = p_const.tile([D, D], f16)
            b1_t = p_const.tile([128, 1], f32)
            b2_t = p_const.tile([128, 1], f32)
            iota_t = p_const.tile([128, 128], f16)
            ident_t = p_const.tile([128, 128], f16)
            dcol_t = p_const.tile([128, 1], f32)
            snorm_t = p_const.tile([128, NBLK], f32)
            dloc_t = p_const.tile([128, t_total], f32)
            norm_t = p_const.tile([128, t_total], f32)
            idx_t = p_const.tile([128, t_total * 8], i16)
            xT_t = p_const.tile([D, SHARD], f16)
            relu1_t = p_big.tile([128, SHARD], f16, tag="relu1")
            for tt, dd in ((W1_t, W1_d), (W2_t, W2_d), (b1_t, b1_d),
                           (b2_t, b2_d), (iota_t, iota_d), (ident_t, ident_d),
                           (dcol_t, dcol_d), (snorm_t, snorm_d),
                           (dloc_t, dloc_d), (norm_t, norm_d), (idx_t, idx_d),
                           (xT_t, xT_d)):
                nc.sync.dma_start(out=tt[:], in_=dd[:])

            def build_S(out_ap, scalar1, scalar2):
                nc.vector.tensor_scalar(
                    out=out_ap, in0=iota_t[:], scalar1=scalar1, scalar2=scalar2,
                    op0=mybir.AluOpType.is_equal, op1=mybir.AluOpType.mult)

            def dense_block(lhsT_full, W_t, hloc_t, l, j):
                ps = p_dps.tile([128, D], f32, space="PSUM", tag="dps")
                nc.tensor.matmul(out=ps[:],
                                 lhsT=lhsT_full[:, j * 128:(j + 1) * 128],
                                 rhs=W_t[:], start=True, stop=True)
                hsl = hloc_t[:, j * 128:(j + 1) * 128]
                nc.scalar.activation(out=hsl, in_=ps[:],
                                     func=mybir.ActivationFunctionType.Copy)
                q, jr = j // JPQ, j % JPQ
                nc.sync.dma_start(out=h_bq[l][q][jr * 128:(jr + 1) * 128, :],
                                  in_=hsl)
                if (j + 1) % JPQ == 0 and not SKIP_CC:
                    nc.gpsimd.collective_compute(
                        "AllGather", mybir.AluOpType.bypass,
                        replica_groups=[list(range(NCORES))],
                        ins=[h_bq[l][q][:]], outs=[h_w[l][q][:]])

            msg0 = None
            if SKIP_GATHER:
                msg0 = p_const.tile([128, CHUNK_T, D], f16)
                nc.vector.memset(msg0[:], 0.5)

            def edge_phase(l, hloc_t, epilogue_cb):
                # superblock-major: self + all windows accumulate into one
                # persistent PSUM bank per superblock, then epilogue reads it
                call_i = 0
                for s in range(NSB):
                    pw = p_wps.tile([128, SB * 128], f32, space="PSUM",
                                    tag="wps")
                    # zero the bank, then every matmul accumulates: a
                    # start=True on a bank wipes other OPEN groups in it
                    nc.vector.memset(pw[:], 0.0)
                    for bi in range(SB):
                        b = s * SB + bi
                        Ssf = p_sel.tile([128, 128], f16, tag="sel")
                        build_S(Ssf[:], dcol_t[:, :1], snorm_t[:, b:b + 1])
                        nc.tensor.matmul(
                            out=pw[:, bi * 128:(bi + 1) * 128],
                            lhsT=hloc_t[:, b * 128:(b + 1) * 128], rhs=Ssf[:],
                            start=False, stop=SKIP_EDGE,
                            skip_group_check=SGC)
                    for w in range(NW):
                        while (call_i < len(calls) and calls[call_i][1] == s
                               and calls[call_i][0] == w):
                            _, _, c0, cnt = calls[call_i]
                            if SKIP_GATHER:
                                msg = msg0
                            else:
                                msg = p_msg.tile([128, CHUNK_T, D], f16,
                                                 tag="msg")
                                nc.gpsimd.dma_gather(
                                    out_ap=msg[:, :cnt, :], in_ap=h_w[l][w][:],
                                    idxs_ap=idx_t[:, c0 * 8:(c0 + cnt) * 8],
                                    num_idxs=cnt * 128, num_idxs_reg=cnt * 128,
                                    elem_size=D, queue_num=call_i % 4)
                            call_i += 1
                            if SKIP_EDGE:
                                continue
                            for t in range(c0, c0 + cnt):
                                bi_t, _, stop_t = tinfo[t]
                                St = p_sel.tile([128, 128], f16, tag="sel")
                                build_S(St[:], dloc_t[:, t:t + 1],
                                        norm_t[:, t:t + 1])
                                nc.tensor.matmul(
                                    out=pw[:, bi_t * 128:(bi_t + 1) * 128],
                                    lhsT=msg[:, t - c0, :], rhs=St[:],
                                    start=False, stop=stop_t,
                                    skip_group_check=SGC)
                    epilogue_cb(s, pw)

            # ---------- layer 1 ----------
            hloc_t = p_big.tile([128, SHARD], f16, tag="hloc")
            for j in range(NBLK):
                dense_block(xT_t, W1_t, hloc_t, 0, j)

            hloc2_t = None

            def epi1(s, pw):
                nc.scalar.activation(
                    out=relu1_t[:, s * 512:(s + 1) * 512], in_=pw[:],
                    func=mybir.ActivationFunctionType.Relu, bias=b1_t[:, :1])
                for bi in range(SB):
                    b = s * SB + bi
                    dense_block(relu1_t, W2_t, hloc2_t, 1, b)

            hloc2_t = p_big.tile([128, SHARD], f16, tag="hloc2")
            edge_phase(0, hloc_t, epi1)

            # ---------- layer 2 ----------
            def epi2(s, pw):
                r16 = p_r16.tile([128, 512], f16, tag="r16")
                nc.scalar.activation(
                    out=r16[:], in_=pw[:],
                    func=mybir.ActivationFunctionType.Relu, bias=b2_t[:, :1])
                for bi in range(SB):
                    b = s * SB + bi
                    tp = p_tps.tile([128, 128], f16, space="PSUM", tag="tps")
                    nc.tensor.transpose(tp[:], r16[:, bi * 128:(bi + 1) * 128],
                                        ident_t[:])
                    o32 = p_o32.tile([128, 128], f32, tag="o32")
                    nc.scalar.activation(out=o32[:], in_=tp[:],
                                         func=mybir.ActivationFunctionType.Copy)
                    nc.sync.dma_start(out=out_d[b * 128:(b + 1) * 128, :],
                                      in_=o32[:])

            edge_phase(1, hloc2_t, epi2)

            if DEBUG_DUMP:
                nc.gpsimd.dma_start(out=dbg_r1[:], in_=relu1_t[:])
                dt2 = p_o32.tile([128, D], f32, tag="o32")
                nc.scalar.activation(out=dt2[:], in_=hloc_t[:, :128],
                                     func=mybir.ActivationFunctionType.Copy)
                nc.sync.dma_start(out=dbg_h1[:, :128], in_=dt2[:])
                wt = p_o32.tile([128, D], f16, tag="wtile")
                nc.sync.dma_start(out=wt[:], in_=h_w[0][0][:128, :])
                wt2 = p_o32.tile([128, D], f32, tag="o32")
                nc.scalar.activation(out=wt2[:], in_=wt[:],
                                     func=mybir.ActivationFunctionType.Copy)
                nc.sync.dma_start(out=dbg_w[:], in_=wt2[:])

    nc.compile()
    return nc


def prepare(x, edge_index, W1, b1, W2, b2):
    (in_maps, sched_sig, tile_base, t_total, tinfo, ws_range, calls,
     pos_of_node) = _host_prep(x, edge_index, W1, b1, W2, b2)
    key = (sched_sig, SKIP_CC, SKIP_GATHER, SKIP_EDGE, DEBUG_DUMP, SGC)
    if key not in _CACHE:
        _CACHE[key] = _build_program(tile_base, t_total, tinfo, ws_range,
                                     calls)
    return _CACHE[key], in_maps, pos_of_node


def kernel(x, edge_index, W1, b1, W2, b2):
    nc, in_maps, pos_of_node = prepare(x, edge_index, W1, b1, W2, b2)
    res = run_bass_kernel_spmd(nc, in_maps, list(range(NCORES)))
    full = np.concatenate([res.results[k]["out"] for k in range(NCORES)], axis=0)
    n = np.asarray(x).shape[0]
    return full[pos_of_node[:n]]
